# revision 17
# baseline (speedup 1.0000x reference)
"""Trainium2 Bass kernel for MecFormer MoE-routing block.

Computation (per sample b):
  h1 = relu(hs @ gw0.T + gb0); h2 = relu(h1 @ gw1.T + gb1)
  logits = h2 @ gw2.T + gb2
  gate = softmax(logits).mean(tokens)            -> output 2
  coef = gate + ohe
  W_eff = W_base + sum_e coef[e] dW[e];  b_eff = b_base + coef @ db
  out  = relu(hs @ W_eff.T + b_eff)              -> output 1

Sharding: data-parallel over batch, 8 samples per core, weights replicated.
All activations kept feature-major [f partitions, tokens] on SBUF (PE
transposes); matmuls run in float32r (full PE rate, fp32 storage).
"""

import numpy as np
from contextlib import ExitStack

import concourse.bass as bass
import concourse.mybir as mybir
import concourse.tile as tile
from concourse import bass_isa
from concourse.bass_utils import run_bass_kernel_spmd
from concourse.masks import make_identity

F32 = mybir.dt.float32
F32R = mybir.dt.float32r
AF = mybir.ActivationFunctionType
ALU = mybir.AluOpType
AX = mybir.AxisListType

B, S, F, D, E = 64, 1024, 1024, 512, 5
NCORES = 8
BPC = B // NCORES


def build_program(bpc=BPC, s=S):
    f, d, e = F, D, E
    NF = f // 128            # f tiles (contraction)
    CH = min(512, s)         # tokens per chunk (matmul moving dim)
    NCH = s // CH            # chunks per sample
    TPC = CH // 128          # 128-token tiles per chunk

    nc = bass.Bass()
    hs = nc.declare_dram_parameter("hidden_states", [bpc, s, f], F32, isOutput=False)
    ohe = nc.declare_dram_parameter("ohe_task", [bpc, e], F32, isOutput=False)
    gw0 = nc.declare_dram_parameter("gw0", [f, f], F32, isOutput=False)
    gb0 = nc.declare_dram_parameter("gb0", [f], F32, isOutput=False)
    gw1 = nc.declare_dram_parameter("gw1", [f, f], F32, isOutput=False)
    gb1 = nc.declare_dram_parameter("gb1", [f], F32, isOutput=False)
    gw2 = nc.declare_dram_parameter("gw2", [e, f], F32, isOutput=False)
    gb2 = nc.declare_dram_parameter("gb2", [e], F32, isOutput=False)
    wb = nc.declare_dram_parameter("W_base", [d, f], F32, isOutput=False)
    bb = nc.declare_dram_parameter("b_base", [d], F32, isOutput=False)
    dw = nc.declare_dram_parameter("dW", [e, d, f], F32, isOutput=False)
    db = nc.declare_dram_parameter("db", [e, d], F32, isOutput=False)
    out = nc.declare_dram_parameter("out", [bpc, s, d], F32, isOutput=True)
    gate_out = nc.declare_dram_parameter("gate_weights", [bpc, e], F32, isOutput=True)

    # dW (and W_base in slot E) pre-transposed to [f, d], tiled [slot, fi, 128, d]
    dwt_dram = nc.dram_tensor("dwt_scratch", [e + 1, NF, 128, d], F32)

    def r32(ap):
        return ap.bitcast(F32R)

    with tile.TileContext(nc) as tc, ExitStack() as ctx:
        const = ctx.enter_context(tc.tile_pool(name="const", bufs=1))
        wres = ctx.enter_context(tc.tile_pool(name="wres", bufs=1))
        hstage = ctx.enter_context(tc.tile_pool(name="hstage", bufs=2))
        x0p = ctx.enter_context(tc.tile_pool(name="x0", bufs=3))
        h1p = ctx.enter_context(tc.tile_pool(name="h1", bufs=1))
        h2p = ctx.enter_context(tc.tile_pool(name="h2", bufs=2))
        weffp = ctx.enter_context(tc.tile_pool(name="weff", bufs=1))
        dwtp = ctx.enter_context(tc.tile_pool(name="dwt", bufs=5))
        outp = ctx.enter_context(tc.tile_pool(name="outs", bufs=2))
        smallp = ctx.enter_context(tc.tile_pool(name="small", bufs=2))

        trps = ctx.enter_context(tc.tile_pool(name="trps", bufs=2, space="PSUM"))
        mmps = ctx.enter_context(tc.tile_pool(name="mmps", bufs=2, space="PSUM"))
        lgps = ctx.enter_context(tc.tile_pool(name="lgps", bufs=1, space="PSUM"))
        gateps = ctx.enter_context(tc.tile_pool(name="gateps", bufs=1, space="PSUM"))
        outps = ctx.enter_context(tc.tile_pool(name="outps", bufs=2, space="PSUM"))

        # ---- constants ----
        ident = const.tile([128, 128], F32)
        make_identity(nc, ident)
        ones_row = const.tile([1, 128], F32)
        nc.vector.memset(ones_row, 1.0)
        one1 = const.tile([1, 1], F32)
        nc.vector.memset(one1, 1.0)
        ones_col = const.tile([e, 1], F32)
        nc.vector.memset(ones_col, 1.0)
        ones_row_r = const.tile([1, 128], F32)
        nc.vector.tensor_copy(ones_row_r.bitcast(F32R), ones_row)

        gb0sb = const.tile([128, NF], F32)
        nc.sync.dma_start(out=gb0sb, in_=gb0.rearrange("(n p) -> p n", p=128))
        gb1sb = const.tile([128, NF], F32)
        nc.sync.dma_start(out=gb1sb, in_=gb1.rearrange("(n p) -> p n", p=128))
        gb2col = const.tile([e, 1], F32)
        nc.sync.dma_start(out=gb2col, in_=gb2.rearrange("(e a) -> e a", a=1))
        bbrow = const.tile([1, d], F32)
        nc.sync.dma_start(out=bbrow, in_=bb.rearrange("(a d) -> a d", a=1))
        dbsb = const.tile([e, d], F32)
        nc.sync.dma_start(out=dbsb, in_=db[:, :])
        oheT = const.tile([e, bpc], F32)
        nc.sync.dma_start(out=oheT, in_=ohe.rearrange("b e -> e b"))

        # ---- transpose square router weights into SBUF (feature-major) ----
        def transpose_square(src, dst):
            # src DRAM [rows=f, cols=f] -> dst SBUF tile [128, NF(fi), f(rows)]
            for ri in range(NF):
                row = hstage.tile([128, f], F32, tag="hstage")
                nc.sync.dma_start(out=row, in_=src[ri * 128:(ri + 1) * 128, :])
                for half in range(NF // 4):
                    pt = trps.tile([128, 4, 128], F32, tag="trps")
                    for j in range(4):
                        fi = half * 4 + j
                        nc.tensor.transpose(
                            pt[:, j, :], row[:, fi * 128:(fi + 1) * 128], ident
                        )
                    nc.vector.tensor_copy(
                        dst[:, half * 4:half * 4 + 4, ri * 128:(ri + 1) * 128]
                        .bitcast(F32R),
                        pt,
                    )

        gw0T = wres.tile([128, NF, f], F32)
        transpose_square(gw0, gw0T)
        gw1T = wres.tile([128, NF, f], F32)
        transpose_square(gw1, gw1T)

        # gw2 [e, f] -> gw2T [128, fi, e]
        gw2T = wres.tile([128, NF, e], F32)
        g2row = hstage.tile([e, f], F32, tag="g2row")
        nc.sync.dma_start(out=g2row, in_=gw2[:, :])
        for half in range(NF // 4):
            pt = trps.tile([128, 4, 128], F32, tag="trps")
            for j in range(4):
                fi = half * 4 + j
                nc.tensor.transpose(
                    pt[:, j, :e], g2row[:, fi * 128:(fi + 1) * 128], ident[:e, :e]
                )
            nc.vector.tensor_copy(gw2T[:, half * 4:half * 4 + 4, :].bitcast(F32R), pt[:, :, :e])

        # ---- transpose dW / W_base into dwt_dram [slot, fi, 128, d] ----
        for slot in range(e + 1):
            src2d = dw[slot] if slot < e else wb[:, :]  # [d, f]
            stage = weffp.tile([128, NF, d], F32, tag="weff")
            for dt in range(d // 128):
                row = hstage.tile([128, f], F32, tag="hstage")
                nc.sync.dma_start(out=row, in_=src2d[dt * 128:(dt + 1) * 128, :])
                for half in range(NF // 4):
                    pt = trps.tile([128, 4, 128], F32, tag="trps")
                    for j in range(4):
                        fi = half * 4 + j
                        nc.tensor.transpose(
                            pt[:, j, :], row[:, fi * 128:(fi + 1) * 128], ident
                        )
                    nc.vector.tensor_copy(
                        stage[:, half * 4:half * 4 + 4, dt * 128:(dt + 1) * 128], pt
                    )
            nc.sync.dma_start(
                out=dwt_dram[slot].rearrange("fi p dc -> p fi dc"), in_=stage
            )

        # ---- main per-sample pipeline ----
        for b in range(bpc):
            gate_acc = None
            x0_chunks = []
            for c in range(NCH):
                x0c = x0p.tile([128, NF, CH], F32, tag="x0")
                x0_chunks.append(x0c)
                # load + transpose hidden states chunk
                for tt in range(TPC):
                    row = hstage.tile([128, f], F32, tag="hstage")
                    nc.sync.dma_start(
                        out=row,
                        in_=hs[b, c * CH + tt * 128: c * CH + (tt + 1) * 128, :],
                    )
                    for half in range(NF // 4):
                        pt = trps.tile([128, 4, 128], F32, tag="trps")
                        for j in range(4):
                            fi = half * 4 + j
                            nc.tensor.transpose(
                                pt[:, j, :], row[:, fi * 128:(fi + 1) * 128], ident
                            )
                        nc.scalar.activation(
                            x0c[:, half * 4:half * 4 + 4, tt * 128:(tt + 1) * 128]
                            .bitcast(F32R),
                            pt,
                            AF.Copy,
                        )

                # router layer 0
                h1c = h1p.tile([128, NF, CH], F32, tag="h1")
                for hi in range(NF):
                    ps = mmps.tile([128, CH], F32, tag="mm")
                    for fi in range(NF):
                        nc.tensor.matmul(
                            ps,
                            r32(gw0T[:, fi, hi * 128:(hi + 1) * 128]),
                            r32(x0c[:, fi, :]),
                            start=(fi == 0),
                            stop=(fi == NF - 1),
                        )
                    nc.scalar.activation(
                        h1c[:, hi, :].bitcast(F32R), ps, AF.Relu,
                        bias=gb0sb[:, hi:hi + 1],
                    )

                # router layer 1 + expert-major logits [e, CH]
                lgt = lgps.tile([e, CH], F32, tag="lg")
                for gi in range(NF):
                    ps = mmps.tile([128, CH], F32, tag="mm")
                    for hi in range(NF):
                        nc.tensor.matmul(
                            ps,
                            r32(gw1T[:, hi, gi * 128:(gi + 1) * 128]),
                            r32(h1c[:, hi, :]),
                            start=(hi == 0),
                            stop=(hi == NF - 1),
                        )
                    h2t = h2p.tile([128, CH], F32, tag="h2")
                    nc.scalar.activation(
                        h2t.bitcast(F32R), ps, AF.Relu, bias=gb1sb[:, gi:gi + 1]
                    )
                    nc.tensor.matmul(
                        lgt,
                        r32(gw2T[:, gi, :]),
                        r32(h2t),
                        start=(gi == 0),
                        stop=(gi == NF - 1),
                    )

                # softmax over experts (partition axis) + token partial-sum
                exps = smallp.tile([e, CH], F32, tag="exps")
                nc.scalar.activation(exps, lgt, AF.Exp, bias=gb2col)
                se_ps = lgps.tile([1, CH], F32, tag="lg")
                nc.tensor.matmul(se_ps, ones_col, exps, start=True, stop=True)
                rinv = smallp.tile([1, CH], F32, tag="rinv")
                nc.vector.reciprocal(rinv, se_ps)
                rb_ps = lgps.tile([e, CH], F32, tag="lg")
                nc.tensor.matmul(
                    rb_ps, ones_row[:, :e], rinv, start=True, stop=True
                )
                p_sb = smallp.tile([e, CH], F32, tag="psb")
                nc.vector.tensor_mul(p_sb, exps, rb_ps)
                part = smallp.tile([e, 1], F32, tag="part")
                nc.vector.reduce_sum(part, p_sb, axis=AX.X)
                if c == 0:
                    gate_acc = smallp.tile([e, 1], F32, tag="gacc")
                    nc.vector.tensor_copy(gate_acc, part)
                else:
                    nc.vector.tensor_add(gate_acc, gate_acc, part)

            # ---- gate -> coef ----
            gate_sb = smallp.tile([e, 1], F32, tag="gate_sb")
            nc.vector.tensor_scalar_mul(gate_sb, gate_acc, 1.0 / s)
            nc.sync.dma_start(
                out=gate_out[b:b + 1, :].rearrange("a e -> e a"), in_=gate_sb
            )
            coef_sb = smallp.tile([e, 1], F32, tag="coef_sb")
            nc.vector.tensor_add(coef_sb, gate_sb, oheT[:, b:b + 1])

            cps = gateps.tile([1, e], F32, tag="gate")
            nc.tensor.transpose(cps, coef_sb, ident[:e, :e])
            coef_row = smallp.tile([1, e], F32, tag="coef_row")
            nc.vector.tensor_copy(coef_row, cps)
            bps = gateps.tile([128, e], F32, tag="gate")
            nc.tensor.matmul(bps, ones_row, coef_row, start=True, stop=True)
            coef_bc = smallp.tile([128, e], F32, tag="coef_bc")
            nc.vector.tensor_copy(coef_bc, bps)

            beps = gateps.tile([1, d], F32, tag="gate")
            nc.tensor.matmul(beps, coef_sb, dbsb, start=True, stop=False)
            nc.tensor.matmul(beps, one1, bbrow, start=False, stop=True)
            beff_row = smallp.tile([1, d], F32, tag="beff")
            nc.vector.tensor_copy(beff_row.bitcast(F32R), beps)

            # ---- merge W_eff (DVE + GPSIMD split) ----
            weff = weffp.tile([128, NF, d], F32, tag="weff")
            for fi in range(NF):
                base_t = dwtp.tile([128, d], F32, tag="dwt")
                nc.sync.dma_start(out=base_t, in_=dwt_dram[e, fi])
                eng = nc.vector
                prev = base_t
                for ei in range(e):
                    dtile = dwtp.tile([128, d], F32, tag="dwt")
                    nc.sync.dma_start(out=dtile, in_=dwt_dram[ei, fi])
                    eng.scalar_tensor_tensor(
                        out=weff[:, fi, :].bitcast(F32R),
                        in0=dtile,
                        scalar=coef_bc[:, ei:ei + 1],
                        in1=prev,
                        op0=ALU.mult,
                        op1=ALU.add,
                    )
                    prev = weff[:, fi, :]

            # ---- main GEMM out = relu(X0.T @ W_effT + b_eff) ----
            for t in range(s // 128):
                cc, ttc = t // TPC, t % TPC
                ps = outps.tile([128, d], F32, tag="ops")
                for fi in range(NF):
                    nc.tensor.matmul(
                        ps,
                        r32(x0_chunks[cc][:, fi, ttc * 128:(ttc + 1) * 128]),
                        r32(weff[:, fi, :]),
                        start=(fi == 0),
                        stop=False,
                    )
                nc.tensor.matmul(
                    ps, r32(ones_row_r), r32(beff_row), start=False, stop=True
                )
                osb = outp.tile([128, d], F32, tag="osb")
                nc.scalar.activation(osb, ps, AF.Relu)
                nc.sync.dma_start(
                    out=out[b, t * 128:(t + 1) * 128, :], in_=osb
                )

    _split_matmul_waits(nc)
    return nc


def _split_matmul_waits(nc, max_waits=1):
    """This walrus build accepts at most one sync wait per TPB instruction.
    Keep the last wait on each instruction and move the rest onto a chain
    of preceding single-wait NoOps on the same engine (in-order dispatch
    makes waits on the NoOps gate the instruction)."""
    fn = nc.m.functions[0]
    for blk in fn.blocks:
        insts = blk.instructions
        out, moved = [], 0
        for inst in insts:
            si = inst.sync_info
            tname = type(inst).__name__
            if (
                si is not None
                and len(si.on_wait) > max_waits
                and tname not in ("InstEventSemaphore", "InstNoOp")
            ):
                waits = list(si.on_wait)
                for w in waits[:-max_waits]:
                    nop = mybir.InstNoOp(
                        name=f"I-waitsplit-{moved}-{inst.name}",
                        engine=inst.engine,
                        ins=[],
                        outs=[],
                    )
                    nop.sync_info = mybir.SyncInfo(on_wait=[w], on_update=[])
                    nc.register_instruction(nop)
                    out.append(nop)
                    moved += 1
                inst.sync_info = mybir.SyncInfo(
                    on_wait=waits[-max_waits:], on_update=list(si.on_update)
                )
            out.append(inst)
        if moved:
            blk.instructions = out


_prog_cache = {}


def _get_prog(bpc=BPC, s=S):
    key = (bpc, s)
    if key not in _prog_cache:
        _prog_cache[key] = build_program(bpc, s)
    return _prog_cache[key]


def kernel(**inputs):
    hs = np.ascontiguousarray(np.asarray(inputs["hidden_states"], dtype=np.float32))
    ohe = np.ascontiguousarray(np.asarray(inputs["ohe_task"], dtype=np.float32))
    shared = {
        k: np.ascontiguousarray(np.asarray(inputs[k], dtype=np.float32))
        for k in ("gw0", "gb0", "gw1", "gb1", "gw2", "gb2", "W_base", "b_base", "dW", "db")
    }
    nc = _get_prog()
    in_maps = []
    for c in range(NCORES):
        m = dict(shared)
        m["hidden_states"] = hs[c * BPC:(c + 1) * BPC]
        m["ohe_task"] = ohe[c * BPC:(c + 1) * BPC]
        in_maps.append(m)
    res = run_bass_kernel_spmd(nc, in_maps, list(range(NCORES)))
    outs = np.concatenate([res.results[i]["out"] for i in range(NCORES)], axis=0)
    gates = np.concatenate(
        [res.results[i]["gate_weights"] for i in range(NCORES)], axis=0
    )
    return outs, gates
